# revision 3
# baseline (speedup 1.0000x reference)
"""Trainium2 Bass kernel for nn_CompressedMoE_31550829757014.

The reference's router/top-k computation is dead code -- the output is just
    out = x @ expert_w[0].T + expert_b[0]
i.e. one (8192 x 2048) x (2048 x 2048) GEMM with a bias.

Strategy:
  * Data-parallel over tokens: 8192 tokens / 8 cores = 1024 tokens per core.
  * Single-pass bf16 matmul with fp32 PSUM accumulation (~2.4e-3 rel RMS err,
    well inside the 2e-2 gate). Per-core roofline: 2*1024*2048*2048 FLOP /
    78.6 TF/s = 109 us.
  * k-major loop over PAIRs of 128-token tiles (2 m x 4 n = 8 PSUM banks):
    the first pair's compute window (~28 us) covers the full 8 MB weight
    stream (~23 us at 358 GB/s), so the PE never waits on W after startup.
  * ~26 warmup matmuls on a memset tile run while the first DMAs are in
    flight, so the PE's HAM clock-gate is released (1.2 -> 2.4 GHz) before
    real compute starts.
  * x0/x1 are loaded in 128 KB k-chunks interleaved into the W stream so the
    first matmul only waits on ~256 KB, and no single large transfer sits
    between consecutively-needed W slices (the previous version lost 3.4 us
    to exactly that).
  * Last group runs n-major so PSUM banks drain progressively; only one
    bias-add + one 256 KB store remain after the final matmul.
"""

import numpy as np
import ml_dtypes

BF16 = ml_dtypes.bfloat16

B, S, D, E = 4, 2048, 2048, 8
N_CORES = 8
T_CORE = (B * S) // N_CORES  # 1024 tokens per core


def _build_nc(T, DD, O, n_tile=512, n_warmup=26):
    """Build the per-core Bass program: out[T,O] = xT.T @ w0T + bias.

    DRAM params (per core):
      xh   : [T/128, 128, DD/128, 128]  x-shard transposed + pre-tiled
             ([m,p,k,t] = xT[k*128+p, m*128+t]), bf16
      wh   : [DD, O]  W0.T bf16 (replicated across cores)
      bias : [128, O] f32  b0 broadcast to 128 partitions (replicated)
      out  : [T, O]   f32
    """
    import concourse.bacc as bacc
    import concourse.mybir as mybir
    import concourse.tile as tile
    from concourse.bass import ts

    P = 128
    KT = DD // P          # 16 contraction tiles
    MT = T // P           # 8 token tiles
    NT = O // n_tile      # 4 output-feature tiles
    PAIR = 2              # m-tiles per psum group; PAIR*NT = 8 PSUM banks
    CH = 4                # k-slices per x0/x1 DMA chunk (128 KB)

    nc = bacc.Bacc(
        "TRN2", target_bir_lowering=False, debug=False, num_devices=N_CORES
    )
    f32 = mybir.dt.float32
    bf16 = mybir.dt.bfloat16

    xh = nc.declare_dram_parameter("xh", [MT, P, KT, P], bf16, isOutput=False)
    wh = nc.declare_dram_parameter("wh", [DD, O], bf16, isOutput=False)
    bias = nc.declare_dram_parameter("bias", [P, O], f32, isOutput=False)
    out = nc.declare_dram_parameter("out", [T, O], f32, isOutput=True)

    wh_r = wh.rearrange("(k p) o -> p k o", p=P)

    with tile.TileContext(nc) as tc:
        with (
            tc.tile_pool(name="wpool", bufs=1) as wpool,
            tc.tile_pool(name="xcpool", bufs=2) as xcpool,
            tc.tile_pool(name="xpool", bufs=MT - 2) as xpool,
            tc.tile_pool(name="opool", bufs=8) as opool,
            tc.tile_pool(name="psum", bufs=8, space="PSUM") as psum,
        ):
            # --- HAM warmup: keep the PE busy while the first DMAs land so
            # the clock-gate releases (~3.4 us of sustained activity) before
            # real matmuls start. Zero-filled operands, result discarded.
            warm_w = wpool.tile([P, P], bf16, tag="warm")
            nc.vector.memset(warm_w[:], 0)
            warm_ps = psum.tile([P, n_tile], f32, tag="ps", name="warm")
            for _ in range(n_warmup):
                nc.tensor.matmul(
                    warm_ps[:, 0:P], warm_w[:], warm_w[:], start=True, stop=True
                )

            # --- DMA loads ---------------------------------------------------
            x_chunks = {}

            def load_xc(m, j):
                t = xcpool.tile([P, CH, P], bf16, tag=f"xc{j}", name=f"x{m}c{j}")
                nc.sync.dma_start(t[:], xh[m][:, CH * j : CH * (j + 1)])
                x_chunks[(m, j)] = t

            x_sb = {}

            def load_x(m):
                t = xpool.tile([P, KT, P], bf16, tag="x", name=f"x_{m}")
                nc.sync.dma_start(t[:], xh[m])
                x_sb[m] = t

            wh_sb = [None] * KT

            def load_wh(k):
                t = wpool.tile([P, O], bf16, tag=f"wh{k}", name=f"wh{k}")
                nc.sync.dma_start(t[:], wh_r[:, k])
                wh_sb[k] = t

            # k=0's W slice is split so the first matmul waits on only 256 KB.
            w0a = wpool.tile([P, n_tile], bf16, tag="w0a")
            w0b = wpool.tile([P, O - n_tile], bf16, tag="w0b")

            # Issue order is arrival order. Per-k needs: W slice k (512 KB)
            # plus, every CH steps, one 128 KB chunk each of x0/x1. Demand
            # pace (1.73 us/k-step warm) slightly exceeds supply pace
            # (~1.55 us/k-step), so the stream stays just ahead after the
            # first step. bias is only read by the first copyback (~30 us);
            # x2..x7 aren't needed until their group starts.
            load_xc(0, 0)
            nc.sync.dma_start(w0a[:], wh_r[:, 0, 0:n_tile])
            load_xc(1, 0)
            nc.sync.dma_start(w0b[:], wh_r[:, 0, n_tile:O])
            load_wh(1)
            load_wh(2)
            load_xc(0, 1)
            load_xc(1, 1)
            for k in range(3, 7):
                load_wh(k)
            load_xc(0, 2)
            load_xc(1, 2)
            for k in range(7, 11):
                load_wh(k)
            load_xc(0, 3)
            load_xc(1, 3)
            for k in range(11, KT):
                load_wh(k)
            b_sb = wpool.tile([P, O], f32, tag="bias")
            nc.sync.dma_start(b_sb[:], bias[:])
            for m in range(2, MT):
                load_x(m)

            # --- compute -----------------------------------------------------
            def xop(m, k):
                if m < 2:
                    return x_chunks[(m, k // CH)][:, k % CH]
                return x_sb[m][:, k]

            def rhs(k, n):
                if k == 0:
                    if n == 0:
                        return w0a[:]
                    return w0b[:, ts(n - 1, n_tile)]
                return wh_sb[k][:, ts(n, n_tile)]

            def copyback(ps_t, m, n):
                ob = opool.tile([P, n_tile], f32, tag="ob", name=f"ob_{m}_{n}")
                nc.vector.tensor_add(
                    out=ob[:], in0=ps_t[:], in1=b_sb[:, ts(n, n_tile)]
                )
                nc.sync.dma_start(out[ts(m, P), ts(n, n_tile)], ob[:])

            n_groups = MT // PAIR
            for g in range(n_groups):
                ms = list(range(g * PAIR, (g + 1) * PAIR))
                ps = {
                    (m, n): psum.tile([P, n_tile], f32, tag="ps", name=f"ps_{m}_{n}")
                    for m in ms
                    for n in range(NT)
                }
                if g < n_groups - 1:
                    # k-major: all 8 banks accumulate in parallel; copyback
                    # emitted right behind each bank's final matmul.
                    for k in range(KT):
                        last = k == KT - 1
                        for m in ms:
                            for n in range(NT):
                                nc.tensor.matmul(
                                    ps[(m, n)][:],
                                    xop(m, k),
                                    rhs(k, n),
                                    start=(k == 0),
                                    stop=last,
                                )
                                if last:
                                    copyback(ps[(m, n)], m, n)
                else:
                    # Last group n-major: banks finish staggered through the
                    # window, so after the very last matmul only one add and
                    # one 256 KB store remain.
                    for n in range(NT):
                        for k in range(KT):
                            for m in ms:
                                nc.tensor.matmul(
                                    ps[(m, n)][:],
                                    xop(m, k),
                                    rhs(k, n),
                                    start=(k == 0),
                                    stop=(k == KT - 1),
                                )
                        for m in ms:
                            copyback(ps[(m, n)], m, n)

    nc.compile()
    return nc


def _tile_xT(xt_2d):
    """[D, T] -> [T//128, 128, D//128, 128] with [m,p,k,t] = xt[k*128+p, m*128+t]."""
    DD, T = xt_2d.shape
    return np.ascontiguousarray(
        xt_2d.reshape(DD // 128, 128, T // 128, 128).transpose(2, 1, 0, 3)
    )


def _prep_in_maps(x, expert_w, expert_b):
    x2 = np.asarray(x, dtype=np.float32).reshape(B * S, D)
    w0t = np.ascontiguousarray(np.asarray(expert_w, dtype=np.float32)[0].T)  # [D, O]
    wh = w0t.astype(BF16)
    bias = np.ascontiguousarray(
        np.broadcast_to(np.asarray(expert_b, dtype=np.float32)[0], (128, D)).astype(
            np.float32
        )
    )
    in_maps = []
    for c in range(N_CORES):
        xct = x2[c * T_CORE : (c + 1) * T_CORE].T  # [D, T] view
        in_maps.append(
            {
                "xh": _tile_xT(xct.astype(BF16)),
                "wh": wh,
                "bias": bias,
            }
        )
    return in_maps


_NC_CACHE = {}


def kernel(x, router_w, expert_w, expert_b):
    from concourse.bass_utils import run_bass_kernel_spmd

    in_maps = _prep_in_maps(x, expert_w, expert_b)
    if "nc" not in _NC_CACHE:
        _NC_CACHE["nc"] = _build_nc(T_CORE, D, D)
    nc = _NC_CACHE["nc"]
    res = run_bass_kernel_spmd(nc, in_maps, list(range(N_CORES)))
    outs = [res.results[c]["out"] for c in range(N_CORES)]
    full = np.concatenate(outs, axis=0).reshape(B, S, D)
    return np.ascontiguousarray(full.astype(np.float32))


# revision 5
# speedup vs baseline: 1.1967x; 1.1967x over previous
"""Trainium2 Bass kernel for nn_CompressedMoE_31550829757014.

The reference's router/top-k computation is dead code -- the output is just
    out = x @ expert_w[0].T + expert_b[0]
i.e. one (8192 x 2048) x (2048 x 2048) GEMM with a bias.

Strategy:
  * Data-parallel over tokens: 8192 tokens / 8 cores = 1024 tokens per core.
  * Single-pass bf16 matmul with fp32 PSUM accumulation (~2.4e-3 rel RMS err,
    well inside the 2e-2 gate). Per-core roofline: 2*1024*2048*2048 FLOP /
    78.6 TF/s = 109 us.
  * k-major loop over PAIRs of 128-token tiles (2 m x 4 n = 8 PSUM banks):
    the first pair's compute window (~28 us) covers the full 8 MB weight
    stream (~23 us at 358 GB/s), so the PE never waits on W after startup.
  * ~26 warmup matmuls on a memset tile run while the first DMAs are in
    flight, so the PE's HAM clock-gate is released (1.2 -> 2.4 GHz) before
    real compute starts.
  * x0/x1 are loaded in 128 KB k-chunks interleaved into the W stream so the
    first matmul only waits on ~256 KB, and no single large transfer sits
    between consecutively-needed W slices (the previous version lost 3.4 us
    to exactly that).
  * Last group runs n-major so PSUM banks drain progressively; only one
    bias-add + one 256 KB store remain after the final matmul.
"""

import numpy as np
import ml_dtypes

BF16 = ml_dtypes.bfloat16

B, S, D, E = 4, 2048, 2048, 8
N_CORES = 8
T_CORE = (B * S) // N_CORES  # 1024 tokens per core


def _build_nc(T, DD, O, n_tile=512, n_warmup=52):
    """Build the per-core Bass program: out[T,O] = xT.T @ w0T + bias.

    DRAM params (per core):
      xh   : [T/128, 128, DD/128, 128]  x-shard transposed + pre-tiled
             ([m,p,k,t] = xT[k*128+p, m*128+t]), bf16
      wh   : [DD, O]  W0.T bf16 (replicated across cores)
      bias : [1, O] f32  b0 row (broadcast to 128 partitions on device)
      out  : [T, O]   f32
    """
    import concourse.bacc as bacc
    import concourse.mybir as mybir
    import concourse.tile as tile
    from concourse.bass import ts

    P = 128
    KT = DD // P          # 16 contraction tiles
    MT = T // P           # 8 token tiles
    NT = O // n_tile      # 4 output-feature tiles
    PAIR = 2              # m-tiles per psum group; PAIR*NT = 8 PSUM banks
    # x0/x1 k-chunk boundaries: small first chunk so the first matmul waits
    # on as little data as possible.
    CHB = [0, 2, 6, 11, KT]

    nc = bacc.Bacc(
        "TRN2", target_bir_lowering=False, debug=False, num_devices=N_CORES
    )
    f32 = mybir.dt.float32
    bf16 = mybir.dt.bfloat16

    xh = nc.declare_dram_parameter("xh", [MT, P, KT, P], bf16, isOutput=False)
    wh = nc.declare_dram_parameter("wh", [DD, O], bf16, isOutput=False)
    bias = nc.declare_dram_parameter("bias", [1, O], f32, isOutput=False)
    out = nc.declare_dram_parameter("out", [T, O], f32, isOutput=True)

    wh_r = wh.rearrange("(k p) o -> p k o", p=P)

    with tile.TileContext(nc) as tc:
        with (
            tc.tile_pool(name="wpool", bufs=1) as wpool,
            tc.tile_pool(name="xcpool", bufs=2) as xcpool,
            tc.tile_pool(name="xpool", bufs=MT - 2) as xpool,
            tc.tile_pool(name="opool", bufs=8) as opool,
            tc.tile_pool(name="psum", bufs=8, space="PSUM") as psum,
        ):
            # --- HAM warmup: keep the PE busy while the first DMAs land so
            # the clock-gate releases (~3.4 us of sustained activity) before
            # real compute, and so the PE has work until the first W chunk's
            # DMA semaphore fires (~6.5 us after program start: descriptor
            # gen + transfer + ~3 us completion receipt).
            warm_w = wpool.tile([P, P], bf16, tag="warm")
            nc.vector.memset(warm_w[:], 0)
            warm_ps = psum.tile([P, n_tile], f32, tag="ps", name="warm")
            for _ in range(n_warmup):
                nc.tensor.matmul(
                    warm_ps[:, 0:P], warm_w[:], warm_w[:], start=True, stop=True
                )

            # --- DMA loads ---------------------------------------------------
            # The tiny pace-critical transfers (x0/x1 first chunks, bias row)
            # go on the scalar-engine HWDGE queue: its descriptor generation
            # runs in parallel with the sync queue's, and its 136 KB drains
            # immediately, leaving the sync queue's bandwidth to the W stream.
            x_chunks = {}

            def load_xc(m, j, eng):
                lo, hi = CHB[j], CHB[j + 1]
                t = xcpool.tile([P, hi - lo, P], bf16, tag=f"xc{j}", name=f"x{m}c{j}")
                eng.dma_start(t[:], xh[m][:, lo:hi])
                x_chunks[(m, j)] = t

            x_sb = {}

            def load_x(m):
                t = xpool.tile([P, KT, P], bf16, tag="x", name=f"x_{m}")
                nc.sync.dma_start(t[:], xh[m])
                x_sb[m] = t

            wh_sb = [None] * KT

            def load_wh(k):
                t = wpool.tile([P, O], bf16, tag=f"wh{k}", name=f"wh{k}")
                nc.sync.dma_start(t[:], wh_r[:, k])
                wh_sb[k] = t

            # k=0's W slice is split so the first matmul waits on only ~192 KB.
            w0a = wpool.tile([P, n_tile], bf16, tag="w0a")
            w0b = wpool.tile([P, O - n_tile], bf16, tag="w0b")

            load_xc(0, 0, nc.scalar)
            load_xc(1, 0, nc.scalar)
            b_row = wpool.tile([1, O], f32, tag="biasrow")
            nc.scalar.dma_start(b_row[:], bias[:])
            b_sb = wpool.tile([P, O], f32, tag="bias")
            nc.gpsimd.partition_broadcast(b_sb[:], b_row[:])

            # Sync-queue issue order is arrival order, matched to consumption
            # order: w0 (split), then W slices interleaved with the remaining
            # x0/x1 chunks one step ahead of their k-step. Demand (1.73 us per
            # k-step warm) slightly exceeds supply (~1.55 us/k-step), so after
            # the first step the stream stays ahead. x2..x7 aren't needed
            # until their group starts (>40 us).
            nc.sync.dma_start(w0a[:], wh_r[:, 0, 0:n_tile])
            nc.sync.dma_start(w0b[:], wh_r[:, 0, n_tile:O])
            load_wh(1)
            load_xc(0, 1, nc.sync)
            load_xc(1, 1, nc.sync)
            load_wh(2)
            load_wh(3)
            load_xc(0, 2, nc.sync)
            load_xc(1, 2, nc.sync)
            for k in range(4, 7):
                load_wh(k)
            load_xc(0, 3, nc.sync)
            load_xc(1, 3, nc.sync)
            for k in range(7, KT):
                load_wh(k)
            for m in range(2, MT):
                load_x(m)

            # --- compute -----------------------------------------------------
            def xop(m, k):
                if m < 2:
                    for j in range(len(CHB) - 1):
                        if CHB[j] <= k < CHB[j + 1]:
                            return x_chunks[(m, j)][:, k - CHB[j]]
                return x_sb[m][:, k]

            def rhs_cols(k, c0, c1):
                if k == 0:
                    if c1 <= n_tile:
                        return w0a[:, c0:c1]
                    return w0b[:, c0 - n_tile : c1 - n_tile]
                return wh_sb[k][:, c0:c1]

            def copyback(ps_ap, m, c0, c1, name):
                ob = opool.tile([P, c1 - c0], f32, tag=f"ob{c1 - c0}", name=name)
                nc.vector.tensor_add(out=ob[:], in0=ps_ap, in1=b_sb[:, c0:c1])
                nc.sync.dma_start(out[ts(m, P), c0:c1], ob[:])

            n_groups = MT // PAIR
            for g in range(n_groups):
                ms = list(range(g * PAIR, (g + 1) * PAIR))
                ps = {
                    (m, n): psum.tile([P, n_tile], f32, tag="ps", name=f"ps_{m}_{n}")
                    for m in ms
                    for n in range(NT)
                }
                if g < n_groups - 1:
                    # k-major: all 8 banks accumulate in parallel (so the
                    # first group's compute window covers the whole W
                    # stream); copyback emitted right behind each bank's
                    # final matmul.
                    for k in range(KT):
                        last = k == KT - 1
                        for m in ms:
                            for n in range(NT):
                                nc.tensor.matmul(
                                    ps[(m, n)][:],
                                    xop(m, k),
                                    rhs_cols(k, n * n_tile, (n + 1) * n_tile),
                                    start=(k == 0),
                                    stop=last,
                                )
                                if last:
                                    copyback(
                                        ps[(m, n)][:], m, n * n_tile,
                                        (n + 1) * n_tile, f"ob_{m}_{n}",
                                    )
                else:
                    # Last group: one (m,n) bank at a time so banks drain
                    # progressively; the final bank runs as two half-width
                    # chains so only a [128,256] add + 128 KB store remain
                    # after the very last matmul.
                    for n in range(NT):
                        for m in ms:
                            c0 = n * n_tile
                            if m == ms[-1] and n == NT - 1:
                                hw = n_tile // 2
                                for h in range(2):
                                    lo, hi = c0 + h * hw, c0 + (h + 1) * hw
                                    for k in range(KT):
                                        nc.tensor.matmul(
                                            ps[(m, n)][:, h * hw : (h + 1) * hw],
                                            xop(m, k),
                                            rhs_cols(k, lo, hi),
                                            start=(k == 0),
                                            stop=(k == KT - 1),
                                        )
                                    copyback(
                                        ps[(m, n)][:, h * hw : (h + 1) * hw],
                                        m, lo, hi, f"obh_{h}",
                                    )
                            else:
                                for k in range(KT):
                                    nc.tensor.matmul(
                                        ps[(m, n)][:],
                                        xop(m, k),
                                        rhs_cols(k, c0, c0 + n_tile),
                                        start=(k == 0),
                                        stop=(k == KT - 1),
                                    )
                                copyback(
                                    ps[(m, n)][:], m, c0, c0 + n_tile,
                                    f"ob_{m}_{n}",
                                )

    nc.compile()
    return nc


def _tile_xT(xt_2d):
    """[D, T] -> [T//128, 128, D//128, 128] with [m,p,k,t] = xt[k*128+p, m*128+t]."""
    DD, T = xt_2d.shape
    return np.ascontiguousarray(
        xt_2d.reshape(DD // 128, 128, T // 128, 128).transpose(2, 1, 0, 3)
    )


def _prep_in_maps(x, expert_w, expert_b):
    x2 = np.asarray(x, dtype=np.float32).reshape(B * S, D)
    w0t = np.ascontiguousarray(np.asarray(expert_w, dtype=np.float32)[0].T)  # [D, O]
    wh = w0t.astype(BF16)
    bias = np.ascontiguousarray(
        np.asarray(expert_b, dtype=np.float32)[0].reshape(1, D)
    )
    in_maps = []
    for c in range(N_CORES):
        xct = x2[c * T_CORE : (c + 1) * T_CORE].T  # [D, T] view
        in_maps.append(
            {
                "xh": _tile_xT(xct.astype(BF16)),
                "wh": wh,
                "bias": bias,
            }
        )
    return in_maps


_NC_CACHE = {}


def kernel(x, router_w, expert_w, expert_b):
    from concourse.bass_utils import run_bass_kernel_spmd

    in_maps = _prep_in_maps(x, expert_w, expert_b)
    if "nc" not in _NC_CACHE:
        _NC_CACHE["nc"] = _build_nc(T_CORE, D, D)
    nc = _NC_CACHE["nc"]
    res = run_bass_kernel_spmd(nc, in_maps, list(range(N_CORES)))
    outs = [res.results[c]["out"] for c in range(N_CORES)]
    full = np.concatenate(outs, axis=0).reshape(B, S, D)
    return np.ascontiguousarray(full.astype(np.float32))


# revision 26
# speedup vs baseline: 1.2046x; 1.0066x over previous
"""Trainium2 Bass kernel for nn_CompressedMoE_31550829757014.

The reference's router/top-k computation is dead code -- the output is just
    out = x @ expert_w[0].T + expert_b[0]
i.e. one (8192 x 2048) x (2048 x 2048) GEMM with a bias.

Strategy:
  * Data-parallel over tokens: 8192 tokens / 8 cores = 1024 tokens per core.
  * Single-pass bf16 matmul with fp32 PSUM accumulation (~2.4e-3 rel RMS err,
    well inside the 2e-2 gate). Per-core roofline: 2*1024*2048*2048 FLOP /
    78.6 TF/s = 109 us.
  * k-major loop over PAIRs of 128-token tiles (2 m x 4 n = 8 PSUM banks):
    the first pair's compute window (~28 us) covers the full 8 MB weight
    stream (~23 us at 358 GB/s), so the PE never waits on W after startup.
  * Warmup matmuls on a memset tile run while the first DMAs are in flight,
    so the PE's HAM clock-gate is released (1.2 -> 2.4 GHz) and the PE has
    work until the first W chunk's DMA semaphore fires (~6 us after program
    start: descriptor gen + transfer + ~3 us completion receipt).
  * x0/x1 are loaded in k-chunks interleaved into the W stream so the first
    matmul only waits on ~200 KB, and no large transfer sits between
    consecutively-needed W slices. The tiny pace-critical loads ride the
    scalar-engine HWDGE queue (parallel descriptor generation).
  * bias comes in as an 8 KB row and is partition-broadcast on device.
  * W streams as 512 KB k-slices while arrival pacing is tight (k<=11, the
    DMA semaphore fires ~3 us after the data lands, so 1 MB granules would
    stall the matmul consuming their first slice), then 1 MB pairs; x2..x7
    and output stores are paired to cut DMA-op count.
  * Last group drains one PSUM bank at a time; the final bank runs as two
    half-width chains on different banks, leaving only a [128,256] add +
    128 KB store after the very last matmul.
"""

import numpy as np
import ml_dtypes

BF16 = ml_dtypes.bfloat16

B, S, D, E = 4, 2048, 2048, 8
N_CORES = 8
T_CORE = (B * S) // N_CORES  # 1024 tokens per core


def _build_nc(T, DD, O, n_tile=512, n_warmup=42):
    """Build the per-core Bass program: out[T,O] = xT.T @ w0T + bias.

    DRAM params (per core):
      xh   : [T/128, 128, DD/128, 128]  x-shard transposed + pre-tiled
             ([m,p,k,t] = xT[k*128+p, m*128+t]), bf16
      wh   : [DD, O]  W0.T bf16 (replicated across cores)
      bias : [1, O] f32  b0 row (broadcast to 128 partitions on device)
      out  : [T, O]   f32
    """
    import concourse.bacc as bacc
    import concourse.mybir as mybir
    import concourse.tile as tile
    from concourse.bass import ts

    P = 128
    KT = DD // P          # 16 contraction tiles
    MT = T // P           # 8 token tiles
    NT = O // n_tile      # 4 output-feature tiles
    PAIR = 2              # m-tiles per psum group; PAIR*NT = 8 PSUM banks
    # x0/x1 k-chunk boundaries: small first chunk so the first matmul waits
    # on as little data as possible.
    CHB = [0, 2, 6, 11, KT]

    nc = bacc.Bacc(
        "TRN2", target_bir_lowering=False, debug=False, num_devices=N_CORES
    )
    f32 = mybir.dt.float32
    bf16 = mybir.dt.bfloat16

    xh = nc.declare_dram_parameter("xh", [MT, P, KT, P], bf16, isOutput=False)
    wh = nc.declare_dram_parameter("wh", [DD, O], bf16, isOutput=False)
    bias = nc.declare_dram_parameter("bias", [1, O], f32, isOutput=False)
    out = nc.declare_dram_parameter("out", [T, O], f32, isOutput=True)

    wh_r = wh.rearrange("(k p) o -> p k o", p=P)

    with tile.TileContext(nc) as tc:
        with (
            tc.tile_pool(name="wpool", bufs=1) as wpool,
            tc.tile_pool(name="xcpool", bufs=2) as xcpool,
            tc.tile_pool(name="xpool", bufs=(MT - 2) // 2) as xpool,
            tc.tile_pool(name="opool", bufs=4) as opool,
            tc.tile_pool(name="psum", bufs=8, space="PSUM") as psum,
        ):
            # --- HAM warmup: keep the PE busy while the first DMAs land so
            # the clock-gate releases (~3.4 us of sustained activity) before
            # real compute, and so the PE has work until the first W chunk's
            # DMA semaphore fires (~6.5 us after program start: descriptor
            # gen + transfer + ~3 us completion receipt).
            warm_w = wpool.tile([P, P], bf16, tag="warm")
            nc.vector.memset(warm_w[:], 0)
            warm_ps = psum.tile([P, n_tile], f32, tag="ps", name="warm")
            for _ in range(n_warmup):
                nc.tensor.matmul(
                    warm_ps[:, 0:P], warm_w[:], warm_w[:], start=True, stop=True
                )

            # --- DMA loads ---------------------------------------------------
            # The tiny pace-critical transfers (x0/x1 first chunks, bias row)
            # go on the scalar-engine HWDGE queue: its descriptor generation
            # runs in parallel with the sync queue's, and its 136 KB drains
            # immediately, leaving the sync queue's bandwidth to the W stream.
            x_chunks = {}

            def load_xc(m, j, eng):
                lo, hi = CHB[j], CHB[j + 1]
                t = xcpool.tile([P, hi - lo, P], bf16, tag=f"xc{j}", name=f"x{m}c{j}")
                eng.dma_start(t[:], xh[m][:, lo:hi])
                x_chunks[(m, j)] = t

            x_sb = {}

            # W k-slices: singles early (fine-grained arrival pacing +
            # earliest possible semaphores), 1 MB pairs later (fewer DMA ops
            # -> shorter end-of-kernel semaphore drain). wh_ap(k) returns the
            # [128, O] SBUF view of slice k.
            wh_sb = {}

            def load_wh(k):
                t = wpool.tile([P, O], bf16, tag=f"wh{k}", name=f"wh{k}")
                nc.sync.dma_start(t[:], wh_r[:, k])
                wh_sb[k] = t[:]

            def load_wh_pair(k):
                t = wpool.tile([P, 2, O], bf16, tag=f"whp{k}", name=f"whp{k}")
                nc.sync.dma_start(t[:], wh_r[:, k : k + 2])
                wh_sb[k] = t[:, 0]
                wh_sb[k + 1] = t[:, 1]

            # k=0's W slice is split at the n-tile boundary (the first matmul
            # waits on only 128 KB); k=1 in halves for an earlier first
            # semaphore. Splits MUST align to PSUM-bank column ranges: two
            # start=True region matmuls in one bank do not compose (the
            # second start clears the whole bank's has_written bits).
            w0a = wpool.tile([P, n_tile], bf16, tag="w0a")
            w0b = wpool.tile([P, O - n_tile], bf16, tag="w0b")
            w1a = wpool.tile([P, O // 2], bf16, tag="w1a")
            w1b = wpool.tile([P, O // 2], bf16, tag="w1b")

            load_xc(0, 0, nc.scalar)
            load_xc(1, 0, nc.scalar)
            b_row = wpool.tile([1, O], f32, tag="biasrow")
            nc.scalar.dma_start(b_row[:], bias[:])
            load_xc(0, 1, nc.scalar)
            load_xc(1, 1, nc.scalar)
            b_sb = wpool.tile([P, O], f32, tag="bias")
            nc.gpsimd.partition_broadcast(b_sb[:], b_row[:])

            # Sync-queue issue order is arrival order, matched to consumption
            # order: w0 (split), then W slices interleaved with the remaining
            # x0/x1 chunks just ahead of their k-step. Demand (1.73 us per
            # k-step warm) slightly exceeds supply (~1.43 us/slice), so after
            # the first steps the stream stays ahead. x2..x7 aren't needed
            # until their group starts (>40 us).
            nc.sync.dma_start(w0a[:], wh_r[:, 0, 0:n_tile])
            nc.sync.dma_start(w0b[:], wh_r[:, 0, n_tile:O])
            nc.sync.dma_start(w1a[:], wh_r[:, 1, 0 : O // 2])
            nc.sync.dma_start(w1b[:], wh_r[:, 1, O // 2 : O])
            load_wh(2)
            load_wh(3)
            load_xc(0, 2, nc.sync)
            load_xc(1, 2, nc.sync)
            load_wh(4)
            load_wh(5)
            load_wh(6)
            load_wh(7)
            load_wh(8)
            load_wh(9)
            load_xc(0, 3, nc.sync)
            load_xc(1, 3, nc.sync)
            load_wh(10)
            load_wh(11)
            load_wh_pair(12)
            load_wh_pair(14)
            for m in range(2, MT, 2):
                t = xpool.tile([P, 2, KT, P], bf16, tag="x", name=f"x_{m}{m+1}")
                nc.sync.dma_start(t[:], xh.rearrange("m p k t -> p m k t")[:, m : m + 2])
                x_sb[m] = t[:, 0]
                x_sb[m + 1] = t[:, 1]

            # --- compute -----------------------------------------------------
            def xop(m, k):
                if m < 2:
                    for j in range(len(CHB) - 1):
                        if CHB[j] <= k < CHB[j + 1]:
                            return x_chunks[(m, j)][:, k - CHB[j]]
                return x_sb[m][:, k]

            def rhs_cols(k, c0, c1):
                if k == 0:
                    if c1 <= n_tile:
                        return w0a[:, c0:c1]
                    return w0b[:, c0 - n_tile : c1 - n_tile]
                if k == 1:
                    if c1 <= O // 2:
                        return w1a[:, c0:c1]
                    return w1b[:, c0 - O // 2 : c1 - O // 2]
                return wh_sb[k][:, c0:c1]

            def w_cuts(k):
                # W tile boundaries; all are n_tile-aligned so emit_mm never
                # splits a matmul within one PSUM bank's column range.
                if k == 0:
                    return (n_tile,)
                if k == 1:
                    return (O // 2,)
                return ()

            def emit_mm(ps_t, ps_off, m, k, c0, c1, start, stop):
                # One matmul per W-tile column sub-range. Cuts are PSUM-bank
                # aligned (w_cuts), so at most one start=True matmul ever
                # targets a given bank per accumulation chain.
                lo = c0
                for b in [x for x in w_cuts(k) if c0 < x < c1] + [c1]:
                    nc.tensor.matmul(
                        ps_t[:, lo - ps_off : b - ps_off],
                        xop(m, k),
                        rhs_cols(k, lo, b),
                        start=start,
                        stop=stop,
                    )
                    lo = b

            n_groups = MT // PAIR
            for g in range(n_groups):
                ms = list(range(g * PAIR, (g + 1) * PAIR))
                ps = {
                    (m, n): psum.tile([P, n_tile], f32, tag="ps", name=f"ps_{m}_{n}")
                    for m in ms
                    for n in range(NT)
                }
                if g < n_groups - 1:
                    # k-major: all 8 banks accumulate in parallel (so the
                    # first group's compute window covers the whole W
                    # stream). Bias-adds are emitted right behind each bank's
                    # final matmul; stores are paired (two banks per 512 KB
                    # DMA) to halve the output DMA count.
                    obp = {}
                    for k in range(KT):
                        last = k == KT - 1
                        for m in ms:
                            for n in range(NT):
                                emit_mm(
                                    ps[(m, n)][:], n * n_tile, m, k,
                                    n * n_tile, (n + 1) * n_tile,
                                    start=(k == 0), stop=last,
                                )
                                if last:
                                    if n % 2 == 0:
                                        obp[m] = opool.tile(
                                            [P, 2 * n_tile], f32, tag="ob",
                                            name=f"ob_{m}_{n // 2}",
                                        )
                                    nc.vector.tensor_add(
                                        out=obp[m][:, (n % 2) * n_tile : (n % 2 + 1) * n_tile],
                                        in0=ps[(m, n)][:],
                                        in1=b_sb[:, ts(n, n_tile)],
                                    )
                                    if n % 2 == 1:
                                        nc.sync.dma_start(
                                            out[ts(m, P), (n - 1) * n_tile : (n + 1) * n_tile],
                                            obp[m][:],
                                        )
                else:
                    # Last group: one (m,n) bank at a time so banks drain
                    # progressively; the final bank runs as two half-width
                    # chains (the second on a different PSUM bank so it
                    # doesn't serialize behind the first's bias-add), leaving
                    # only a [128,256] add + 128 KB store after the very
                    # last matmul.
                    def chain(ps_ap, m, c0, c1, name):
                        for k in range(KT):
                            emit_mm(
                                ps_ap, c0, m, k, c0, c1,
                                start=(k == 0), stop=(k == KT - 1),
                            )
                        ob = opool.tile(
                            [P, c1 - c0], f32, tag=f"obl{c1 - c0}", name=name
                        )
                        nc.vector.tensor_add(
                            out=ob[:], in0=ps_ap, in1=b_sb[:, c0:c1]
                        )
                        nc.sync.dma_start(out[ts(m, P), c0:c1], ob[:])

                    for n in range(NT):
                        for m in ms:
                            c0 = n * n_tile
                            if m == ms[-1] and n == NT - 1:
                                hw = n_tile // 2
                                ps_h1 = psum.tile(
                                    [P, n_tile], f32, tag="ps", name="ps_h1"
                                )
                                chain(ps[(m, n)][:, 0:hw], m, c0, c0 + hw, "obh0")
                                chain(ps_h1[:, 0:hw], m, c0 + hw, c0 + n_tile, "obh1")
                            else:
                                chain(ps[(m, n)][:], m, c0, c0 + n_tile, f"ob_{m}_{n}")

    nc.compile()
    return nc


def _tile_xT(xt_2d):
    """[D, T] -> [T//128, 128, D//128, 128] with [m,p,k,t] = xt[k*128+p, m*128+t]."""
    DD, T = xt_2d.shape
    return np.ascontiguousarray(
        xt_2d.reshape(DD // 128, 128, T // 128, 128).transpose(2, 1, 0, 3)
    )


def _prep_in_maps(x, expert_w, expert_b):
    x2 = np.asarray(x, dtype=np.float32).reshape(B * S, D)
    w0t = np.ascontiguousarray(np.asarray(expert_w, dtype=np.float32)[0].T)  # [D, O]
    wh = w0t.astype(BF16)
    bias = np.ascontiguousarray(
        np.asarray(expert_b, dtype=np.float32)[0].reshape(1, D)
    )
    in_maps = []
    for c in range(N_CORES):
        xct = x2[c * T_CORE : (c + 1) * T_CORE].T  # [D, T] view
        in_maps.append(
            {
                "xh": _tile_xT(xct.astype(BF16)),
                "wh": wh,
                "bias": bias,
            }
        )
    return in_maps


_NC_CACHE = {}


def kernel(x, router_w, expert_w, expert_b):
    from concourse.bass_utils import run_bass_kernel_spmd

    in_maps = _prep_in_maps(x, expert_w, expert_b)
    if "nc" not in _NC_CACHE:
        _NC_CACHE["nc"] = _build_nc(T_CORE, D, D)
    nc = _NC_CACHE["nc"]
    res = run_bass_kernel_spmd(nc, in_maps, list(range(N_CORES)))
    outs = [res.results[c]["out"] for c in range(N_CORES)]
    full = np.concatenate(outs, axis=0).reshape(B, S, D)
    return np.ascontiguousarray(full.astype(np.float32))
